# revision 14
# baseline (speedup 1.0000x reference)
"""DMTetGeometry kernel for Trainium2 (8 NeuronCores, axon).

Split of work:
  - device (8 NC, data-parallel): dense memory-bound passes
      Pass A: per-tet edge extraction -> (vmax, vmin) int32 pairs that view
              as int64 lexicographic sort keys, 6 edges per tet.
      Pass B: per-crossing-edge vertex interpolation (lerp along edge).
  - host (numpy): occupancy decisions, sort/unique/inverse, table lookups,
      compaction — irregular data-dependent work with no efficient device
      mapping on TRN2.

Self-contained: no imports from the problem directory.
"""

import os
import numpy as np

N_CORES = 8

# Marching-tets tables (DMTet formulation).
TRI_TBL = np.array([
    [-1, -1, -1, -1, -1, -1], [1, 0, 2, -1, -1, -1], [4, 0, 3, -1, -1, -1], [1, 4, 2, 1, 3, 4],
    [3, 1, 5, -1, -1, -1], [2, 3, 0, 2, 5, 3], [1, 4, 0, 1, 5, 4], [4, 2, 5, -1, -1, -1],
    [4, 5, 2, -1, -1, -1], [4, 1, 0, 4, 5, 1], [3, 2, 0, 3, 5, 2], [1, 3, 5, -1, -1, -1],
    [4, 1, 2, 4, 3, 1], [3, 0, 4, -1, -1, -1], [2, 0, 1, -1, -1, -1], [-1, -1, -1, -1, -1, -1]],
    dtype=np.int64)
NUM_TRI = np.array([0, 1, 1, 2, 1, 2, 2, 1, 1, 2, 2, 1, 2, 1, 1, 0], dtype=np.int64)
NUM_TETS_TBL = np.array([0, 1, 1, 3, 1, 3, 3, 3, 1, 3, 3, 3, 3, 3, 3, 1], dtype=np.int64)
TET_TBL = np.array([
    [-1, -1, -1, -1, -1, -1, -1, -1, -1, -1, -1, -1], [0, 4, 5, 6, -1, -1, -1, -1, -1, -1, -1, -1],
    [1, 4, 8, 7, -1, -1, -1, -1, -1, -1, -1, -1], [7, 1, 8, 6, 5, 1, 7, 6, 5, 0, 1, 6],
    [2, 5, 7, 9, -1, -1, -1, -1, -1, -1, -1, -1], [4, 0, 6, 7, 9, 0, 7, 6, 7, 0, 9, 2],
    [4, 1, 9, 8, 5, 1, 9, 4, 5, 1, 2, 9], [6, 0, 1, 2, 8, 6, 1, 2, 9, 6, 8, 2],
    [3, 6, 9, 8, -1, -1, -1, -1, -1, -1, -1, -1], [5, 0, 4, 8, 5, 0, 8, 3, 5, 8, 9, 3],
    [1, 4, 7, 3, 4, 7, 6, 3, 9, 6, 7, 3], [0, 1, 5, 3, 5, 1, 9, 3, 5, 1, 7, 9],
    [5, 2, 3, 7, 3, 6, 5, 8, 3, 5, 7, 8], [0, 4, 7, 8, 0, 3, 8, 7, 0, 3, 7, 2],
    [4, 1, 2, 3, 4, 3, 2, 5, 4, 3, 5, 6], [0, 1, 2, 3, -1, -1, -1, -1, -1, -1, -1, -1]],
    dtype=np.int64)
EDGE_A = (0, 0, 0, 1, 1, 2)
EDGE_B = (1, 2, 3, 2, 3, 3)

TRACE = bool(int(os.environ.get("BASS_DMTET_TRACE", "0")))
LAST_STATS = {}

_PROGRAMS = {}  # (name, shape-key) -> compiled Bacc program


def _install_trace_hook():
    """Provide antenv.axon_hooks so run_bass_kernel_spmd trace=True works."""
    import sys
    import types
    if "antenv.axon_hooks" in sys.modules:
        return
    import antenv  # noqa: F401
    mod = types.ModuleType("antenv.axon_hooks")
    _HOOK = [None]
    mod.set_axon_ntff_profile_hook = lambda h: _HOOK.__setitem__(0, h)
    mod.get_axon_ntff_profile_hook = lambda: _HOOK[0]
    sys.modules["antenv.axon_hooks"] = mod
    from trn_agent_boot.trn_boot import _ntff_profile_via_ctypes
    mod.set_axon_ntff_profile_hook(
        _ntff_profile_via_ctypes("/opt/axon/libaxon_pjrt.so"))


PA_CHUNK = 320    # tet columns per chunk (pass A)
PB_CHUNK = 960    # edge columns per chunk (pass B)

# device edge-plane order (chosen so ops fuse into contiguous plane runs):
# planes 0..5  = vmax of edges (0,1),(1,2),(2,3),(0,2),(1,3),(0,3)
# planes 6..11 = vmin of the same edges
# reference BASE_EDGES slot -> device plane
REF_SLOT_TO_PLANE = (0, 3, 5, 1, 4, 2)


def _cleanup_sems(nc, gp, sems, final_waits):
    """Reset semaphores so the NEFF can be re-executed (PJRT warm calls)."""
    for sem, val in final_waits:
        gp.wait_ge(sem, val)
    nums = sorted(s.num for s in sems)
    lo = nums[0]
    hi = nums[-1]
    assert nums == list(range(lo, hi + 1))
    gp.dma_reset(range(lo, hi + 1))
    gp.sem_clear(range(lo, hi + 1))


NSLOT = 3


def _build_pass_a(ca):
    """Per-tet edge extraction, planar layout, fused plane-run ops (raw bacc).

    in:  tets  [128, ca*4] int32, per partition [nch, 4, PA_CHUNK]
         (planes v0..v3 per chunk)
    out: ekeys [128, ca*12] int32, per partition [nch, 12, PA_CHUNK]
    """
    import concourse.bacc as bacc
    import concourse.mybir as mybir
    from concourse.alu_op_type import AluOpType

    assert ca % PA_CHUNK == 0
    nch = ca // PA_CHUNK
    cw = PA_CHUNK
    nc = bacc.Bacc("TRN2", debug=False, num_devices=N_CORES)
    tets = nc.dram_tensor("tets", [128, ca * 4], mybir.dt.int32, kind="ExternalInput")
    ekeys = nc.dram_tensor("ekeys", [128, ca * 12], mybir.dt.int32, kind="ExternalOutput")
    tap = tets.ap()
    kap = ekeys.ap()

    ins = [nc.alloc_sbuf_tensor(f"ain{j}", [128, 4, cw], mybir.dt.int32)
           for j in range(NSLOT)]
    outs = [nc.alloc_sbuf_tensor(f"aout{j}", [128, 12, cw], mybir.dt.int32)
            for j in range(NSLOT)]
    s_in = nc.alloc_semaphore("s_in")
    s_out = nc.alloc_semaphore("s_out")
    s_cmp = nc.alloc_semaphore("s_cmp")

    with nc.Block() as block:
        @block.sync
        def _(sync):
            for i in range(nch):
                if i >= NSLOT:
                    sync.wait_ge(s_cmp, i - NSLOT + 1)
                sync.dma_start(ins[i % NSLOT].ap(),
                               tap[:, i * 4 * cw:(i + 1) * 4 * cw]).then_inc(s_in, 16)

        @block.scalar
        def _(scalar):
            for i in range(nch):
                scalar.wait_ge(s_cmp, i + 1)
                scalar.dma_start(kap[:, i * 12 * cw:(i + 1) * 12 * cw],
                                 outs[i % NSLOT].ap()).then_inc(s_out, 16)

        @block.vector
        def _(vector):
            for i in range(nch):
                t = ins[i % NSLOT].ap()
                o = outs[i % NSLOT].ap()
                vector.wait_ge(s_in, 16 * (i + 1))
                if i >= NSLOT:
                    vector.wait_ge(s_out, 16 * (i - NSLOT + 1))
                last = None
                for base, op in ((0, AluOpType.max), (6, AluOpType.min)):
                    vector.tensor_tensor(
                        o[:, base:base + 3, :], t[:, 0:3, :], t[:, 1:4, :], op)
                    vector.tensor_tensor(
                        o[:, base + 3:base + 5, :], t[:, 0:2, :], t[:, 2:4, :], op)
                    last = vector.tensor_tensor(
                        o[:, base + 5, :], t[:, 0, :], t[:, 3, :], op)
                last.then_inc(s_cmp, 1)

        @block.gpsimd
        def _(gp):
            _cleanup_sems(nc, gp, [s_in, s_out, s_cmp],
                          [(s_out, 16 * nch)])
    nc.compile()
    return nc


def _build_pass_b(cb):
    """Crossing-edge interpolation (lerp), planar layout (raw bacc).

    in:  edata [128, cb*7] f32, per partition [nch, 7, PB_CHUNK]
         (planes: pax,pay,paz,pbx,pby,pbz,t)
    out: verts [128, cb*3] f32, per partition [nch, 3, PB_CHUNK]
    verts = pa + t*(pb - pa)   (t precomputed on host)
    """
    import concourse.bacc as bacc
    import concourse.mybir as mybir
    from concourse.alu_op_type import AluOpType

    assert cb % PB_CHUNK == 0
    nch = cb // PB_CHUNK
    cw = PB_CHUNK
    nc = bacc.Bacc("TRN2", debug=False, num_devices=N_CORES)
    edata = nc.dram_tensor("edata", [128, cb * 7], mybir.dt.float32, kind="ExternalInput")
    verts = nc.dram_tensor("verts", [128, cb * 3], mybir.dt.float32, kind="ExternalOutput")
    eap = edata.ap()
    vap = verts.ap()

    ins = [nc.alloc_sbuf_tensor(f"bin{j}", [128, 7, cw], mybir.dt.float32)
           for j in range(NSLOT)]
    outs = [nc.alloc_sbuf_tensor(f"bout{j}", [128, 3, cw], mybir.dt.float32)
            for j in range(NSLOT)]
    s_in = nc.alloc_semaphore("s_in")
    s_out = nc.alloc_semaphore("s_out")
    s_cmp = nc.alloc_semaphore("s_cmp")

    with nc.Block() as block:
        @block.sync
        def _(sync):
            for i in range(nch):
                if i >= NSLOT:
                    sync.wait_ge(s_cmp, i - NSLOT + 1)
                sync.dma_start(ins[i % NSLOT].ap(),
                               eap[:, i * 7 * cw:(i + 1) * 7 * cw]).then_inc(s_in, 16)

        @block.scalar
        def _(scalar):
            for i in range(nch):
                scalar.wait_ge(s_cmp, i + 1)
                scalar.dma_start(vap[:, i * 3 * cw:(i + 1) * 3 * cw],
                                 outs[i % NSLOT].ap()).then_inc(s_out, 16)

        @block.vector
        def _(vector):
            for i in range(nch):
                t = ins[i % NSLOT].ap()
                o = outs[i % NSLOT].ap()
                vector.wait_ge(s_in, 16 * (i + 1))
                if i >= NSLOT:
                    vector.wait_ge(s_out, 16 * (i - NSLOT + 1))
                tb = t[:, 6:7, :].broadcast_to([128, 3, cw])
                vector.tensor_tensor(o[:], t[:, 3:6, :], t[:, 0:3, :],
                                     AluOpType.subtract)
                vector.tensor_tensor(o[:], o[:], tb, AluOpType.mult)
                vector.tensor_tensor(o[:], o[:], t[:, 0:3, :],
                                     AluOpType.add).then_inc(s_cmp, 1)

        @block.gpsimd
        def _(gp):
            _cleanup_sems(nc, gp, [s_in, s_out, s_cmp],
                          [(s_out, 16 * nch)])
    nc.compile()
    return nc


def _get_program(name, key, builder):
    k = (name, key)
    if k not in _PROGRAMS:
        _PROGRAMS[k] = builder(key)
    return _PROGRAMS[k]


def _run_spmd(nc, in_maps, label):
    from concourse import bass_utils
    if TRACE:
        _install_trace_hook()
        import tempfile
        tmpdir = tempfile.mkdtemp(prefix=f"dmtet_{label}_")
        res = bass_utils.run_bass_kernel_spmd(
            nc, in_maps, core_ids=list(range(N_CORES)), trace=True, tmpdir=tmpdir,
            trace_cores=[0])
        LAST_STATS[label] = {
            "exec_time_ns": res.exec_time_ns,
            "mean_exec_time_ns": res.mean_exec_time_ns,
            "trace": res.instructions_and_trace[1] if res.instructions_and_trace else None,
            "tmpdir": tmpdir,
        }
        return res.results
    res = bass_utils.run_bass_kernel_spmd(nc, in_maps, core_ids=list(range(N_CORES)))
    return res.results


def _unique_with_inverse(keys):
    order = np.argsort(keys, kind="stable")
    sk = keys[order]
    new_flag = np.empty(sk.shape[0], dtype=bool)
    new_flag[0] = True
    np.not_equal(sk[1:], sk[:-1], out=new_flag[1:])
    ukeys = sk[new_flag]
    ranks = np.cumsum(new_flag) - 1
    inverse = np.empty_like(order)
    inverse[order] = ranks
    return ukeys, inverse


def kernel(pos, sdf1, sdf2, interp_coef, tet):
    n = pos.shape[0]
    f = tet.shape[0]
    pos = np.ascontiguousarray(pos, dtype=np.float32)
    sdf1 = np.ascontiguousarray(sdf1, dtype=np.float32)
    sdf2 = np.ascontiguousarray(sdf2, dtype=np.float32)
    c = np.float32(np.asarray(interp_coef))
    tet = np.asarray(tet)

    sdf = c * sdf1 + (np.float32(1.0) - c) * sdf2
    occ = sdf > 0

    # ---------------- device pass A: per-tet edge keys ----------------
    fs = (f + N_CORES - 1) // N_CORES             # tets per shard
    ca = -(-fs // 128)                            # columns per partition
    ca = -(-ca // PA_CHUNK) * PA_CHUNK            # pad to chunk multiple
    ncha = ca // PA_CHUNK
    fsp = ca * 128                                # padded shard size
    tet32 = np.zeros((N_CORES * fsp, 4), dtype=np.int32)
    tet32[:f] = tet.reshape(f, 4)
    # per-shard planar layout [128, nch, 4(comp), PA_CHUNK]
    tp = np.ascontiguousarray(
        tet32.reshape(N_CORES, 128, ncha, PA_CHUNK, 4).transpose(0, 1, 2, 4, 3))

    nc_a = _get_program("passA", ca, _build_pass_a)
    in_maps = [{"tets": tp[s].reshape(128, ca * 4)} for s in range(N_CORES)]
    res_a = _run_spmd(nc_a, in_maps, "passA")

    # reassemble 6 int64 key planes in tet order
    keys_e = np.empty((6, N_CORES * fsp), dtype=np.int64)
    for s in range(N_CORES):
        r = res_a[s]["ekeys"].reshape(128, ncha, 12, PA_CHUNK)
        for e in range(6):
            p = REF_SLOT_TO_PLANE[e]
            vmax = r[:, :, p, :].astype(np.int64)
            vmin = r[:, :, 6 + p, :].astype(np.int64)
            keys_e[e, s * fsp:(s + 1) * fsp] = ((vmin << 32) | vmax).reshape(fsp)

    # ---------------- host: occupancy, valid tets, unique edges ----------------
    occ_f = occ[tet]                              # [F,4]
    occ_sum = occ_f.sum(-1)
    valid = (occ_sum > 0) & (occ_sum < 4)
    vt = tet[valid]                               # [T,4] int64
    occ_v = occ_f[valid]

    # plane-major edge stream: slot (e, t) at index e*T + t
    kv = np.concatenate([keys_e[e, :f][valid] for e in range(6)])
    ukeys, idx_map = _unique_with_inverse(kv)
    ua = (ukeys >> 32).astype(np.int64)           # [E]
    ub = (ukeys & 0xFFFFFFFF).astype(np.int64)
    mask_edges = occ[ua] != occ[ub]               # crossing
    mapping = np.where(mask_edges, np.cumsum(mask_edges) - 1, -1)
    idx_map = mapping[idx_map]
    a = ua[mask_edges]
    b = ub[mask_edges]
    m = a.shape[0]

    # ---------------- device pass B: interpolation ----------------
    ms = (m + N_CORES - 1) // N_CORES
    cb = -(-ms // 128)
    cb = -(-cb // PB_CHUNK) * PB_CHUNK            # chunk multiple (program reuse)
    nchb = cb // PB_CHUNK
    msp = cb * 128
    mp = N_CORES * msp
    pa_g = np.zeros((mp, 3), dtype=np.float32)
    pa_g[:m] = pos[a]
    pb_g = np.zeros((mp, 3), dtype=np.float32)
    pb_g[:m] = pos[b]
    sa = sdf[a]
    sb = sdf[b]
    t_g = np.zeros(mp, dtype=np.float32)
    t_g[:m] = sa / (sa - sb)
    # planar layout [128, nch, 7(plane), PB_CHUNK] per shard
    edata = np.empty((N_CORES, 128, nchb, 7, PB_CHUNK), dtype=np.float32)
    shp = (N_CORES, 128, nchb, PB_CHUNK)
    for d in range(3):
        edata[:, :, :, d, :] = pa_g[:, d].reshape(shp)
        edata[:, :, :, 3 + d, :] = pb_g[:, d].reshape(shp)
    edata[:, :, :, 6, :] = t_g.reshape(shp)

    nc_b = _get_program("passB", cb, _build_pass_b)
    in_maps = [{"edata": edata[s].reshape(128, cb * 7)} for s in range(N_CORES)]
    res_b = _run_spmd(nc_b, in_maps, "passB")

    verts = np.empty((mp, 3), dtype=np.float32)
    for s in range(N_CORES):
        r = res_b[s]["verts"].reshape(128, nchb, 3, PB_CHUNK)
        for d in range(3):
            verts[s * msp:(s + 1) * msp, d] = r[:, :, d, :].reshape(msp)
    verts = verts[:m]

    # ---------------- host: faces / side tets / final mesh ----------------
    idx_map6 = np.ascontiguousarray(idx_map.reshape(6, -1).T)
    tetindex = (occ_v[:, 0].astype(np.int64) + 2 * occ_v[:, 1] + 4 * occ_v[:, 2]
                + 8 * occ_v[:, 3])
    ntri = NUM_TRI[tetindex]
    tri = TRI_TBL[tetindex]
    m1 = ntri == 1
    m2 = ntri == 2
    f1 = np.take_along_axis(idx_map6[m1], tri[m1][:, :3], axis=1).reshape(-1, 3)
    f2 = np.take_along_axis(idx_map6[m2], tri[m2][:, :6], axis=1).reshape(-1, 3)
    faces = np.concatenate([f1, f2], axis=0)

    ntet = NUM_TETS_TBL[tetindex]
    tve = np.concatenate([vt, idx_map6 + n], axis=1)
    tt = TET_TBL[tetindex]
    s1 = ntet == 1
    s3 = ntet == 3
    t1 = np.take_along_axis(tve[s1], tt[s1][:, :4], axis=1).reshape(-1, 4)
    t3 = np.take_along_axis(tve[s3], tt[s3][:, :12], axis=1).reshape(-1, 4)
    side_tets = np.concatenate([t1, t3], axis=0)

    inner_tets = tet[occ_sum == 4]
    all_tets = np.concatenate([side_tets, inner_tets], axis=0)
    flat = all_tets.reshape(-1)
    present = np.zeros(n + m, dtype=bool)
    present[flat] = True
    u = np.flatnonzero(present)
    lut = np.cumsum(present) - 1
    inv = lut[flat]
    all_tets_tetmesh = inv.reshape(-1, 4)

    all_verts = np.concatenate([pos, verts], axis=0)
    all_verts_tetmesh = all_verts[u]
    return verts, faces, all_verts_tetmesh, all_tets_tetmesh
